# revision 1
# baseline (speedup 1.0000x reference)
"""Trainium2 Bass kernel for KNN-masked multi-head agent-agent attention.

Problem (per scene): N=1024 agents, D=256 model dim, H=4 heads, K=32 nearest
neighbours by distance. Full pipeline:
    top-K mask from distances -> additive bias (-d/50, -inf outside mask)
    -> MHA (shared in-proj, softmax, out-proj) -> residual + LayerNorm.

Sharding: data-parallel over the batch axis B=8 -> one scene per NeuronCore
(8 cores), no collectives. Each core runs the identical program (SPMD) on its
own scene; the host stacks per-core outputs.

Per-core algorithm:
  * selection: 4 rounds of max8 + match_replace on nd=-d give the exact
    top-32 values per query row; T = 32nd smallest distance (negated).
    Boundary ties (d_(32) == d_(33)) are broken by index like jax.lax.top_k:
    a fused gpsimd pass computes s1 = sum((nd==T) * (1024+idx)) from which the
    allowed-index cutoff I is derived in closed form (valid for <=2-way ties,
    which is verified to hold with huge margin for uniform random distances).
  * bias_nat[q, k] = [d <= T_q] * (128 - d/50), tie-killed entries -> 0.
  * attention in transposed layout: S^T = K_h Q_h^T accumulated in PSUM,
    bias_nat transpose-accumulated into the same PSUM tile by the PE,
    probs = ACT exp(PSUM - 128) (the +128 offset makes masked entries
    underflow to exactly 0), AV^T with a ones-augmented V so the softmax
    denominator falls out of the same matmul, late normalization.
  * out-proj back to natural layout, residual + LayerNorm epilogue (bn_stats).
"""

import os
import sys
import numpy as np

sys.path.insert(0, "/opt/trn_rl_repo")

import concourse.bass as bass
import concourse.tile as tile
from concourse import mybir
from concourse.masks import make_identity

f32 = mybir.dt.float32
f32r = mybir.dt.float32r
Alu = mybir.AluOpType
Act = mybir.ActivationFunctionType

N = 1024
D = 256
H = 4
HD = 64
NT = N // 128          # 8 query/token tiles
KB = N // 128          # 8 key blocks
D_REF = 50.0
LN_EPS = 1e-5
NEG_BIG = -1.0e30
MASK_OFS = 128.0       # exp(S + bias - 128): masked entries underflow to 0

# matmul dtype for the big products (f32r = TF32-rate, 4x faster than f32)
MM_DT = f32r


def build_nc(K: int, split_waits: bool = True):
    nc = bass.Bass("TRN2", target_bir_lowering=False, debug=False)

    x_d = nc.dram_tensor("repr1", [N, D], f32, kind="ExternalInput").ap()
    d_d = nc.dram_tensor("distances", [N, N], f32, kind="ExternalInput").ap()
    wi_d = nc.dram_tensor("in_proj_w", [3 * D, D], f32, kind="ExternalInput").ap()
    bi_d = nc.dram_tensor("in_proj_b", [3 * D], f32, kind="ExternalInput").ap()
    wo_d = nc.dram_tensor("out_proj_w", [D, D], f32, kind="ExternalInput").ap()
    bo_d = nc.dram_tensor("out_proj_b", [D], f32, kind="ExternalInput").ap()
    g_d = nc.dram_tensor("ln_gamma", [D], f32, kind="ExternalInput").ap()
    be_d = nc.dram_tensor("ln_beta", [D], f32, kind="ExternalInput").ap()
    out_d = nc.dram_tensor("out", [N, D], f32, kind="ExternalOutput").ap()
    rden_d = nc.dram_tensor("rden_bounce", [H, N], f32).ap()  # internal bounce

    with tile.TileContext(nc) as tc:
        _emit(tc, K, x_d, d_d, wi_d, bi_d, wo_d, bo_d, g_d, be_d, out_d, rden_d)
    if split_waits:
        _split_waits(nc)
    return nc


def _split_waits(nc, max_waits: int = 1):
    """Walrus codegen rejects instructions carrying more than one sync wait
    (e.g. transpose-matmul LDW structs and HWDGE DMA descriptors). Move the
    extra waits onto engine NoOps issued immediately before — the sequencer
    stalls on those first, which is semantically identical."""
    k = 0
    for fn in nc.m.functions:
        for blk in fn.blocks:
            new = []
            for ins in blk.instructions:
                si = ins.sync_info
                if si is not None and si.on_wait and len(si.on_wait) > max_waits:
                    waits = list(si.on_wait)
                    for w in waits[:-max_waits]:
                        nop = mybir.InstNoOp(
                            name=f"I-wsplit-{k}", engine=ins.engine)
                        nop.sync_info = mybir.SyncInfo(on_wait=[w], on_update=[])
                        new.append(nop)
                        k += 1
                    ins.sync_info = mybir.SyncInfo(
                        on_wait=waits[-max_waits:], on_update=list(si.on_update))
                new.append(ins)
            blk.instructions[:] = new


def _bcast_dram_row(nc, dst, src_ap, offset, width):
    """DMA-replicate a [width] DRAM row into all 128 partitions of dst."""
    rep = bass.AP(
        tensor=src_ap.tensor,
        offset=src_ap.offset + offset,
        ap=[[0, 128], [1, width]],
    )
    nc.gpsimd.dma_start(out=dst, in_=rep)


def _emit(tc, K, x_d, d_d, wi_d, bi_d, wo_d, bo_d, g_d, be_d, out_d, rden_d):
    from contextlib import ExitStack
    nc = tc.nc
    ctx = ExitStack()

    consts = ctx.enter_context(tc.tile_pool(name="consts", bufs=1))
    persist = ctx.enter_context(tc.tile_pool(name="persist", bufs=1))
    dstage = ctx.enter_context(tc.tile_pool(name="dstage", bufs=2))
    selp = ctx.enter_context(tc.tile_pool(name="selp", bufs=2))
    ptp = ctx.enter_context(tc.tile_pool(name="ptp", bufs=10))
    epi = ctx.enter_context(tc.tile_pool(name="epi", bufs=3))
    ps_s = ctx.enter_context(tc.tile_pool(name="ps_s", bufs=3, space="PSUM"))
    ps_av = ctx.enter_context(tc.tile_pool(name="ps_av", bufs=3, space="PSUM"))
    ps_tr = ctx.enter_context(tc.tile_pool(name="ps_tr", bufs=1, space="PSUM"))
    ps_o = ctx.enter_context(tc.tile_pool(name="ps_o", bufs=1, space="PSUM"))

    # ---------------- constants ----------------
    ident = consts.tile([128, 128], f32, name="ident")
    make_identity(nc, ident)
    # PE touches ident once so later transpose-matmuls (which can carry only
    # a single sync wait in walrus codegen) need no wait on the Pool engine.
    identwarm = ps_tr.tile([128, 128], f32, name="identwarm", tag="wtr")
    nc.tensor.matmul(identwarm, lhsT=ident, rhs=ident, is_transpose=True)

    negofs = consts.tile([128, 1], f32, name="negofs")
    nc.vector.memset(negofs, -MASK_OFS)
    epsc = consts.tile([128, 1], f32, name="epsc")
    nc.vector.memset(epsc, LN_EPS)

    iota1 = consts.tile([128, N], f32, name="iota1")  # 1024 + j, exact in f32
    nc.gpsimd.iota(iota1, pattern=[[1, N]], base=N, channel_multiplier=0,
                   allow_small_or_imprecise_dtypes=True)

    # ---------------- weights ----------------
    # W^T for in-proj: [256, 768] as 2 partition tiles of [128, 768]
    wt = [persist.tile([128, 3 * D], f32, name=f"wt{c}") for c in range(2)]
    for r in range(6):  # six [128, 256] row-tiles of in_proj_w
        wrow = dstage.tile([128, D], f32, name="wrow", tag="wrow")
        nc.sync.dma_start(out=wrow, in_=wi_d[r * 128:(r + 1) * 128, :])
        for c in range(2):
            pt = ps_tr.tile([128, 128], f32, name="wtr", tag="wtr")
            nc.tensor.matmul(pt, lhsT=wrow[:, c * 128:(c + 1) * 128], rhs=ident,
                             is_transpose=True)
            nc.scalar.activation(wt[c][:, r * 128:(r + 1) * 128].bitcast(f32r), pt, Act.Copy)
    # fold the attention scale 1/8 into Wq^T (free cols 0..255 = Q features)
    for c in range(2):
        nc.vector.tensor_scalar_mul(wt[c][:, 0:D].bitcast(f32r), wt[c][:, 0:D], 0.125)

    # Wo^T [256, 256] as 2 tiles [128, 256]
    wot = [persist.tile([128, D], f32, name=f"wot{c}") for c in range(2)]
    for r in range(2):
        worow = dstage.tile([128, D], f32, name="worow", tag="wrow")
        nc.sync.dma_start(out=worow, in_=wo_d[r * 128:(r + 1) * 128, :])
        for c in range(2):
            pt = ps_tr.tile([128, 128], f32, name="wotr", tag="wtr")
            nc.tensor.matmul(pt, lhsT=worow[:, c * 128:(c + 1) * 128], rhs=ident,
                             is_transpose=True)
            nc.scalar.activation(wot[c][:, r * 128:(r + 1) * 128].bitcast(f32r), pt, Act.Copy)

    # per-partition in-proj biases for the Q^T/K^T M-blocks (Q biases pre-scaled)
    bqk = []
    for mb in range(4):
        t = consts.tile([128, 1], f32, name=f"bqk{mb}")
        nc.sync.dma_start(out=t, in_=bi_d[mb * 128:(mb + 1) * 128].rearrange(
            "(p o) -> p o", o=1))
        if mb < 2:
            nc.vector.tensor_scalar_mul(t, t, 0.125)
        bqk.append(t)

    bv_b = consts.tile([128, D], f32, name="bv_b")
    _bcast_dram_row(nc, bv_b, bi_d, 2 * D, D)
    bo_b = consts.tile([128, D], f32, name="bo_b")
    _bcast_dram_row(nc, bo_b, bo_d, 0, D)
    g_b = consts.tile([128, D], f32, name="g_b")
    _bcast_dram_row(nc, g_b, g_d, 0, D)
    be_b = consts.tile([128, D], f32, name="be_b")
    _bcast_dram_row(nc, be_b, be_d, 0, D)

    # ---------------- X, Xb, X^T ----------------
    xb = []  # residual + out-proj bias pre-added
    xt = [persist.tile([128, N], f32, name=f"xt{c}") for c in range(2)]
    for i in range(NT):
        xrow = dstage.tile([128, D], f32, name="xrow", tag="wrow")
        (nc.scalar if i % 2 else nc.sync).dma_start(out=xrow, in_=x_d[i * 128:(i + 1) * 128, :])
        for c in range(2):
            pt = ps_tr.tile([128, 128], f32, name="xtr", tag="wtr")
            nc.tensor.matmul(pt, lhsT=xrow[:, c * 128:(c + 1) * 128], rhs=ident,
                             is_transpose=True)
            nc.scalar.activation(xt[c][:, i * 128:(i + 1) * 128].bitcast(f32r), pt, Act.Copy)
        t = persist.tile([128, D], f32, name=f"xb{i}")
        nc.gpsimd.tensor_tensor(t, xrow, bo_b, Alu.add)
        xb.append(t)

    # ---------------- Q^T, K^T, V ----------------
    qkt = [persist.tile([128, N], f32, name=f"qkt{mb}") for mb in range(4)]
    for mb in range(4):
        for qc in range(2):
            ps = ps_s.tile([128, 512], f32, name="qk_ps", tag="ps_s")
            for c in range(2):
                nc.tensor.matmul(
                    ps,
                    lhsT=wt[c][:, mb * 128:(mb + 1) * 128].bitcast(MM_DT),
                    rhs=xt[c][:, qc * 512:(qc + 1) * 512].bitcast(MM_DT),
                    start=(c == 0), stop=(c == 1))
            nc.scalar.activation(qkt[mb][:, qc * 512:(qc + 1) * 512].bitcast(f32r),
                                 ps, Act.Identity, bias=bqk[mb])

    # V padded per head: [128, H, 65]; col 64 of each head slot is the ones
    # column that produces the softmax denominator in the AV matmul.
    vpad = [persist.tile([128, H, HD + 1], f32, name=f"vpad{kb}") for kb in range(KB)]
    ones4 = consts.tile([128, H], f32, name="ones4")
    nc.vector.memset(ones4, 1.0)
    for kb in range(KB):
        nc.vector.tensor_copy(
            vpad[kb][:, :, HD:HD + 1].bitcast(f32r),
            ones4.rearrange("p (h o) -> p h o", o=1))
        ps = ps_o.tile([128, D], f32, name="v_ps", tag="ps_o")
        for c in range(2):
            nc.tensor.matmul(
                ps,
                lhsT=xt[c][:, kb * 128:(kb + 1) * 128].bitcast(MM_DT),
                rhs=wt[c][:, 2 * D:3 * D].bitcast(MM_DT),
                start=(c == 0), stop=(c == 1))
        nc.vector.tensor_tensor(
            vpad[kb][:, :, 0:HD].bitcast(f32r),
            ps.rearrange("p (h e) -> p h e", h=H),
            bv_b.rearrange("p (h e) -> p h e", h=H),
            Alu.add)

    # ---------------- selection + bias build ----------------
    bias_nat = [persist.tile([128, N], f32, name=f"bias{i}") for i in range(NT)]
    s1_all = consts.tile([128, NT], f32, name="s1_all")
    ex_all = consts.tile([128, NT], f32, name="ex_all")
    i_all = consts.tile([128, NT], f32, name="i_all")
    veq_t = {}
    bm_t = {}

    for grp in range(2):          # process query tiles in qc-aligned groups of 4
        tiles = range(grp * 4, grp * 4 + 4)
        for i in tiles:
            drow = dstage.tile([128, N], f32, name="drow", tag="drow")
            dma_eng = nc.sync if i % 2 == 0 else nc.scalar
            dma_eng.dma_start(out=drow, in_=d_d[i * 128:(i + 1) * 128, :])
            nd = selp.tile([128, N], f32, name="nd", tag="nd")
            nc.scalar.activation(nd, drow, Act.Copy, scale=-1.0)  # nd = -d

            m32 = selp.tile([128, 32], f32, name="m32", tag="m32")
            sc = selp.tile([128, N], f32, name="selsc", tag="selsc")
            nc.vector.max(m32[:, 0:8], nd)
            nc.vector.match_replace(sc, m32[:, 0:8], nd, NEG_BIG)
            nc.vector.max(m32[:, 8:16], sc)
            nc.vector.match_replace(sc, m32[:, 8:16], sc, NEG_BIG)
            nc.vector.max(m32[:, 16:24], sc)
            nc.vector.match_replace(sc, m32[:, 16:24], sc, NEG_BIG)
            nc.vector.max(m32[:, 24:32], sc)
            tneg = m32[:, K - 1:K]  # = -d_(K)

            # extra = # of top-(K-1) entries equal to T
            eqw = selp.tile([128, 32], f32, name="eqw", tag="eqw")
            nc.vector.tensor_scalar(eqw[:, 0:K - 1], m32[:, 0:K - 1], tneg, None,
                                    Alu.is_equal)
            nc.vector.reduce_sum(ex_all[:, i:i + 1], eqw[:, 0:K - 1],
                                 axis=mybir.AxisListType.X)

            # veq = (nd == T) * (1024 + idx); s1 = sum(veq)
            veq = selp.tile([128, N], f32, name="veq", tag="veq")
            veq_t[i] = veq
            nc.vector.scalar_tensor_tensor(
                out=veq, in0=nd, scalar=tneg, in1=iota1,
                op0=Alu.is_equal, op1=Alu.mult, accum_out=s1_all[:, i:i + 1])

            # bias_main = 0.02 * nd + 128 * [nd >= T]
            m128 = selp.tile([128, N], f32, name="m128", tag="m128")
            nc.vector.tensor_scalar(m128, nd, tneg, MASK_OFS, Alu.is_ge, Alu.mult)
            bm = selp.tile([128, N], f32, name="bm", tag="bm")
            bm_t[i] = bm
            nc.vector.scalar_tensor_tensor(
                out=bm, in0=nd, scalar=1.0 / D_REF, in1=m128,
                op0=Alu.mult, op1=Alu.add)

            # tie cutoff: e==2 <=> s1 >= 2048; kill iff e==2 and extra==0
            s1i = s1_all[:, i:i + 1]
            is2 = selp.tile([128, 1], f32, name="is2", tag="is2")
            nc.vector.tensor_scalar(is2, s1i, float(2 * N), None, Alu.is_ge)
            ex0 = selp.tile([128, 1], f32, name="ex0", tag="ex0")
            nc.vector.tensor_scalar(ex0, ex_all[:, i:i + 1], 0.5, None, Alu.is_le)
            kf = selp.tile([128, 1], f32, name="kf", tag="kf")
            nc.vector.tensor_tensor(kf, is2, ex0, Alu.mult)
            # I = s1 - kf*(s1+1)/2   (since s1 - (s1-1)/2 == (s1+1)/2)
            u = selp.tile([128, 1], f32, name="u_tie", tag="ikill")
            nc.vector.tensor_scalar(u, s1i, 1.0, 0.5, Alu.add, Alu.mult)
            adj = selp.tile([128, 1], f32, name="adj_tie", tag="idiff")
            nc.vector.tensor_tensor(adj, kf, u, Alu.mult)
            nc.vector.tensor_tensor(i_all[:, i:i + 1], s1i, adj, Alu.subtract)

            # bias = [veq <= I] * bias_main
            nc.vector.scalar_tensor_tensor(
                out=bias_nat[i], in0=veq, scalar=i_all[:, i:i + 1], in1=bm,
                op0=Alu.is_le, op1=Alu.mult)

    # ---------------- attention (transposed layout) ----------------
    # head h: Q^T/K^T rows live in qkt[mb], partitions (h%2)*64 .. +64
    attnt = [persist.tile([128, N], f32, name=f"attnt{c}") for c in range(2)]
    rden = [consts.tile([1, N], f32, name=f"rden{h}") for h in range(H)]
    rb = [persist.tile([128, N], f32, name=f"rb{c}") for c in range(2)]

    # chunk-outer (4 chunks of 256 queries): chunk c needs only bias tiles
    # 2c, 2c+1, so attention starts after two selection tiles and only the
    # last 256-query chunk remains un-overlapped after selection ends.
    CHUNKS = [(0, 512), (512, 256), (768, 256)]
    for q0, QW in CHUNKS:
        qc = q0 // 256  # in 256-units for bias indexing below
        qs = slice(q0, q0 + QW)
        for h in range(H):
            qmb, kmb = h // 2, 2 + h // 2
            p0 = (h % 2) * HD
            pt_tiles = []
            for kb in range(KB):
                ptile = ptp.tile([128, QW], f32, name="pt", tag="pt")
                pt_tiles.append(ptile)
                ps = ps_s.tile([128, QW], f32, name="s_ps", tag="ps_s")
                nc.tensor.matmul(
                    ps,
                    lhsT=qkt[kmb][p0:p0 + HD, kb * 128:(kb + 1) * 128].bitcast(MM_DT),
                    rhs=qkt[qmb][p0:p0 + HD, qs].bitcast(MM_DT),
                    start=True, stop=False)
                for j in range(QW // 128):
                    qb = q0 // 128 + j
                    nc.tensor.matmul(
                        ps[:, j * 128:(j + 1) * 128],
                        lhsT=bias_nat[qb][:, kb * 128:(kb + 1) * 128],
                        rhs=ident, is_transpose=True,
                        start=False, stop=(j == QW // 128 - 1))
                nc.scalar.activation(ptile.bitcast(f32r), ps, Act.Exp,
                                     bias=negofs)
            av = ps_av.tile([HD + 1, QW], f32, name="av_ps", tag="ps_av")
            for kb in range(KB):
                nc.tensor.matmul(
                    av,
                    lhsT=vpad[kb][:, h, :].bitcast(MM_DT),
                    rhs=pt_tiles[kb].bitcast(MM_DT),
                    start=(kb == 0), stop=(kb == KB - 1))
            nc.scalar.activation(
                attnt[h // 2][(h % 2) * HD:(h % 2) * HD + HD, qs].bitcast(f32r),
                av[0:HD, :], Act.Copy)
            nc.vector.reciprocal(rden[h][:, qs], av[HD:HD + 1, :])

        # ---- epilogue for this query chunk (overlaps later chunks' work)
        for h in range(H):
            nc.scalar.dma_start(out=rden_d[h:h + 1, qs], in_=rden[h][:, qs])
        for c in range(2):
            for half in range(2):
                rep = bass.AP(tensor=rden_d.tensor,
                              offset=rden_d.offset + (2 * c + half) * N + q0,
                              ap=[[0, HD], [1, QW]])
                nc.gpsimd.dma_start(out=rb[c][half * HD:(half + 1) * HD, qs],
                                    in_=rep)
            nc.vector.tensor_tensor(attnt[c][:, qs].bitcast(f32r),
                                    attnt[c][:, qs], rb[c][:, qs], Alu.mult)

        for tb in range(q0 // 128, q0 // 128 + QW // 128):
            po = ps_o.tile([128, D], f32, name="o_ps", tag="ps_o")
            for c in range(2):
                nc.tensor.matmul(
                    po,
                    lhsT=attnt[c][:, tb * 128:(tb + 1) * 128].bitcast(MM_DT),
                    rhs=wot[c].bitcast(MM_DT),
                    start=(c == 0), stop=(c == 1))
            x = epi.tile([128, D], f32, name="x_epi", tag="x_epi")
            nc.vector.tensor_tensor(x, po, xb[tb], Alu.add)
            st = epi.tile([128, 6], f32, name="st", tag="st")
            nc.vector.bn_stats(st, x)
            mv = epi.tile([128, 2], f32, name="mv", tag="mv")
            nc.vector.bn_aggr(mv, st)
            sd = epi.tile([128, 1], f32, name="sd", tag="sd")
            nc.scalar.activation(sd, mv[:, 1:2], Act.Sqrt, bias=epsc)
            rstd = epi.tile([128, 1], f32, name="rstd", tag="rstd")
            nc.vector.reciprocal(rstd, sd)
            xc = epi.tile([128, D], f32, name="xc_epi", tag="xc_epi")
            nc.vector.tensor_scalar(xc, x, mv[:, 0:1], None, Alu.subtract)
            y = epi.tile([128, D], f32, name="y_epi", tag="y_epi")
            nc.vector.scalar_tensor_tensor(
                out=y, in0=g_b, scalar=rstd, in1=xc, op0=Alu.mult, op1=Alu.mult)
            nc.gpsimd.tensor_tensor(y, y, be_b, Alu.add)
            (nc.scalar if tb % 2 else nc.sync).dma_start(
                out=out_d[tb * 128:(tb + 1) * 128, :], in_=y)

    ctx.close()


_NC_CACHE = {}


def _get_nc(K: int):
    if K not in _NC_CACHE:
        _NC_CACHE[K] = build_nc(K)
    return _NC_CACHE[K]


def kernel(**inputs) -> np.ndarray:
    from concourse.bass_utils import run_bass_kernel_spmd

    K = int(np.asarray(inputs["K"]))
    assert K == 32, f"kernel specialized for K=32, got {K}"
    B = inputs["repr1"].shape[0]
    nc = _get_nc(K)

    shared = {
        "in_proj_w": np.ascontiguousarray(inputs["in_proj_w"], np.float32),
        "in_proj_b": np.ascontiguousarray(inputs["in_proj_b"], np.float32),
        "out_proj_w": np.ascontiguousarray(inputs["out_proj_w"], np.float32),
        "out_proj_b": np.ascontiguousarray(inputs["out_proj_b"], np.float32),
        "ln_gamma": np.ascontiguousarray(inputs["ln_gamma"], np.float32),
        "ln_beta": np.ascontiguousarray(inputs["ln_beta"], np.float32),
    }
    in_maps = []
    for b in range(B):
        m = dict(shared)
        m["repr1"] = np.ascontiguousarray(inputs["repr1"][b], np.float32)
        m["distances"] = np.ascontiguousarray(inputs["distances"][b], np.float32)
        in_maps.append(m)

    res = run_bass_kernel_spmd(nc, in_maps, list(range(B)))
    out = np.stack([np.asarray(res.results[b]["out"]) for b in range(B)])
    return out.astype(np.float32)



# revision 7
# speedup vs baseline: 1.0920x; 1.0920x over previous
"""Trainium2 Bass kernel for KNN-masked multi-head agent-agent attention.

Problem (per scene): N=1024 agents, D=256 model dim, H=4 heads, K=32 nearest
neighbours by distance. Full pipeline:
    top-K mask from distances -> additive bias (-d/50, -inf outside mask)
    -> MHA (shared in-proj, softmax, out-proj) -> residual + LayerNorm.

Sharding: data-parallel over the batch axis B=8 -> one scene per NeuronCore
(8 cores), no collectives. Each core runs the identical program (SPMD) on its
own scene; the host stacks per-core outputs.

Per-core algorithm:
  * selection: 4 rounds of (max8 + match_replace imm=-1e30) on nd=-d mark the
    exact top-32 multiset in-place: match_replace replaces the lowest-index
    occurrence of each of the 8 values per round, which reproduces
    jax.lax.top_k's index tie-breaking exactly for any tie multiplicity.
    Selected entries of sc equal -1e30 afterwards; everything else keeps nd.
  * bias (negative-offset form, fp16): bias = 0.02*nd + (-44)*[not selected].
    Selected entries carry only 0.02*nd (full fp16 precision); masked entries
    sit near -44 where precision is irrelevant and exp underflows fp16 to 0.
  * attention in transposed layout: S^T = K_h Q_h^T accumulated in PSUM
    (f32r), bias fp16 transpose-accumulated by the PE at 1 cycle/row,
    probs = exp(PSUM) in fp16, AV^T in fp16 with a ones-augmented V so the
    softmax denominator falls out of the same matmul; late normalization via
    a PE selector-matmul broadcast of the reciprocal denominators.
  * out-proj back to natural layout (fp16 weights), residual + LayerNorm
    epilogue (bn_stats on DVE, scale/shift on Pool).
"""

import os
import sys
import numpy as np

sys.path.insert(0, "/opt/trn_rl_repo")

import concourse.bass as bass
import concourse.tile as tile
from concourse import mybir
from concourse.masks import make_identity

f32 = mybir.dt.float32
f32r = mybir.dt.float32r
f16 = mybir.dt.float16
Alu = mybir.AluOpType
Act = mybir.ActivationFunctionType

N = 1024
D = 256
H = 4
HD = 64
NT = N // 128          # 8 query/token tiles
KB = N // 128          # 8 key blocks
D_REF = 50.0
LN_EPS = 1e-5
NEG_BIG = -1.0e30
MASK_M = -44.0         # additive mask for non-selected entries (exp -> 0)

MM_DT = f32r


def build_nc(K: int, split_waits: bool = True):
    nc = bass.Bass("TRN2", target_bir_lowering=False, debug=False)

    x_d = nc.dram_tensor("repr1", [N, D], f32, kind="ExternalInput").ap()
    d_d = nc.dram_tensor("distances", [N, N], f32, kind="ExternalInput").ap()
    wi_d = nc.dram_tensor("in_proj_w", [3 * D, D], f32, kind="ExternalInput").ap()
    bi_d = nc.dram_tensor("in_proj_b", [3 * D], f32, kind="ExternalInput").ap()
    wo_d = nc.dram_tensor("out_proj_w", [D, D], f32, kind="ExternalInput").ap()
    bo_d = nc.dram_tensor("out_proj_b", [D], f32, kind="ExternalInput").ap()
    g_d = nc.dram_tensor("ln_gamma", [D], f32, kind="ExternalInput").ap()
    be_d = nc.dram_tensor("ln_beta", [D], f32, kind="ExternalInput").ap()
    out_d = nc.dram_tensor("out", [N, D], f32, kind="ExternalOutput").ap()

    with tile.TileContext(nc) as tc:
        _emit(tc, K, x_d, d_d, wi_d, bi_d, wo_d, bo_d, g_d, be_d, out_d)
    if split_waits:
        _split_waits(nc)
    return nc


def _split_waits(nc, max_waits: int = 1):
    """Walrus codegen rejects instructions carrying more than one sync wait
    (e.g. transpose-matmul LDW structs and HWDGE DMA descriptors), and the
    DMA_DIRECT2D_XPOSE struct carries none at all. Move the excess waits onto
    engine NoOps issued immediately before — the sequencer stalls on those
    first, which is semantically identical."""
    k = 0
    for fn in nc.m.functions:
        for blk in fn.blocks:
            new = []
            for ins in blk.instructions:
                si = ins.sync_info
                mw = 0 if isinstance(ins, mybir.InstDmaTransposeAnt) else max_waits
                if si is not None and si.on_wait and len(si.on_wait) > mw:
                    waits = list(si.on_wait)
                    keep = waits[-mw:] if mw else []
                    for w in (waits[:-mw] if mw else waits):
                        nop = mybir.InstNoOp(
                            name=f"I-wsplit-{k}", engine=ins.engine)
                        nop.sync_info = mybir.SyncInfo(on_wait=[w], on_update=[])
                        new.append(nop)
                        k += 1
                    ins.sync_info = mybir.SyncInfo(
                        on_wait=keep, on_update=list(si.on_update))
                new.append(ins)
            blk.instructions[:] = new


def _bcast_dram_row(nc, dst, src_ap, offset, width):
    """DMA-replicate a [width] DRAM row into all 128 partitions of dst."""
    rep = bass.AP(
        tensor=src_ap.tensor,
        offset=src_ap.offset + offset,
        ap=[[0, 128], [1, width]],
    )
    nc.gpsimd.dma_start(out=dst, in_=rep)


def _emit(tc, K, x_d, d_d, wi_d, bi_d, wo_d, bo_d, g_d, be_d, out_d):
    from contextlib import ExitStack
    nc = tc.nc
    ctx = ExitStack()

    consts = ctx.enter_context(tc.tile_pool(name="consts", bufs=1))
    persist = ctx.enter_context(tc.tile_pool(name="persist", bufs=1))
    dstage = ctx.enter_context(tc.tile_pool(name="dstage", bufs=3))
    selp = ctx.enter_context(tc.tile_pool(name="selp", bufs=2))
    ptp = ctx.enter_context(tc.tile_pool(name="ptp", bufs=10))
    epi = ctx.enter_context(tc.tile_pool(name="epi", bufs=3))
    ps_s = ctx.enter_context(tc.tile_pool(name="ps_s", bufs=2, space="PSUM"))
    ps_av = ctx.enter_context(tc.tile_pool(name="ps_av", bufs=2, space="PSUM"))
    ps_tr = ctx.enter_context(tc.tile_pool(name="ps_tr", bufs=1, space="PSUM"))
    ps_o = ctx.enter_context(tc.tile_pool(name="ps_o", bufs=2, space="PSUM"))
    ps_rb = ctx.enter_context(tc.tile_pool(name="ps_rb", bufs=1, space="PSUM"))

    # ---------------- constants ----------------
    ident = consts.tile([128, 128], f32, name="ident")
    make_identity(nc, ident)
    identh = consts.tile([128, 128], f16, name="identh")
    nc.vector.tensor_copy(identh, ident)
    # PE touches ident once so later transpose-matmuls (which can carry only
    # a single sync wait in walrus codegen) need no wait on producers.
    identwarm = ps_tr.tile([128, 128], f32, name="identwarm", tag="wtr")
    nc.tensor.matmul(identwarm, lhsT=ident, rhs=ident, is_transpose=True)

    epsc = consts.tile([128, 1], f32, name="epsc")
    nc.vector.memset(epsc, LN_EPS)

    # head-half selector for the denominator broadcast matmul:
    # sel2[0, p] = [p < 64], sel2[1, p] = [p >= 64]
    sel2 = consts.tile([2, 128], f32, name="sel2")
    iota128 = consts.tile([2, 128], f32, name="iota128")
    nc.gpsimd.iota(iota128, pattern=[[1, 128]], base=0, channel_multiplier=0,
                   allow_small_or_imprecise_dtypes=True)
    nc.vector.tensor_scalar(sel2[0:1, :], iota128[0:1, :], float(HD), None,
                            Alu.is_lt)
    nc.vector.tensor_scalar(sel2[1:2, :], iota128[1:2, :], float(HD), None,
                            Alu.is_ge)

    # ---------------- weights ----------------
    # W^T for in-proj: [256, 768] as 2 partition tiles of [128, 768]
    wt = [persist.tile([128, 3 * D], f32, name=f"wt{c}") for c in range(2)]
    for r in range(6):  # six [128, 256] row-tiles of in_proj_w
        wrow = dstage.tile([128, D], f32, name="wrow", tag="wrow")
        nc.sync.dma_start(out=wrow, in_=wi_d[r * 128:(r + 1) * 128, :])
        for c in range(2):
            pt = ps_tr.tile([128, 128], f32, name="wtr", tag="wtr")
            nc.tensor.matmul(pt, lhsT=wrow[:, c * 128:(c + 1) * 128], rhs=ident,
                             is_transpose=True)
            nc.scalar.activation(wt[c][:, r * 128:(r + 1) * 128].bitcast(f32r), pt, Act.Copy)
    # fold the attention scale 1/8 into Wq^T (free cols 0..255 = Q features)
    for c in range(2):
        nc.vector.tensor_scalar_mul(wt[c][:, 0:D].bitcast(f32r), wt[c][:, 0:D], 0.125)

    # Wo^T [256, 256] as 2 fp16 tiles [128, 256]
    wot = [persist.tile([128, D], f16, name=f"wot{c}") for c in range(2)]
    for r in range(2):
        worow = dstage.tile([128, D], f32, name="worow", tag="wrow")
        nc.sync.dma_start(out=worow, in_=wo_d[r * 128:(r + 1) * 128, :])
        for c in range(2):
            pt = ps_tr.tile([128, 128], f32, name="wotr", tag="wtr")
            nc.tensor.matmul(pt, lhsT=worow[:, c * 128:(c + 1) * 128], rhs=ident,
                             is_transpose=True)
            nc.scalar.activation(wot[c][:, r * 128:(r + 1) * 128], pt, Act.Copy)

    # per-partition in-proj biases for the Q^T/K^T M-blocks (Q biases pre-scaled)
    bqk = []
    for mb in range(4):
        t = consts.tile([128, 1], f32, name=f"bqk{mb}")
        nc.sync.dma_start(out=t, in_=bi_d[mb * 128:(mb + 1) * 128].rearrange(
            "(p o) -> p o", o=1))
        if mb < 2:
            nc.vector.tensor_scalar_mul(t, t, 0.125)
        bqk.append(t)

    bv_b = consts.tile([128, D], f32, name="bv_b")
    _bcast_dram_row(nc, bv_b, bi_d, 2 * D, D)
    bo_b = consts.tile([128, D], f32, name="bo_b")
    _bcast_dram_row(nc, bo_b, bo_d, 0, D)
    g_b = consts.tile([128, D], f32, name="g_b")
    _bcast_dram_row(nc, g_b, g_d, 0, D)
    be_b = consts.tile([128, D], f32, name="be_b")
    _bcast_dram_row(nc, be_b, be_d, 0, D)

    # ---------------- X, Xb, X^T ----------------
    xb = []  # residual + out-proj bias pre-added
    xt = [persist.tile([128, N], f32, name=f"xt{c}") for c in range(2)]
    for i in range(NT):
        xrow = dstage.tile([128, D], f32, name="xrow", tag="wrow")
        (nc.scalar if i % 2 else nc.sync).dma_start(out=xrow, in_=x_d[i * 128:(i + 1) * 128, :])
        for c in range(2):
            pt = ps_tr.tile([128, 128], f32, name="xtr", tag="wtr")
            nc.tensor.matmul(pt, lhsT=xrow[:, c * 128:(c + 1) * 128], rhs=ident,
                             is_transpose=True)
            nc.scalar.activation(xt[c][:, i * 128:(i + 1) * 128].bitcast(f32r), pt, Act.Copy)
        t = persist.tile([128, D], f32, name=f"xb{i}")
        nc.gpsimd.tensor_tensor(t, xrow, bo_b, Alu.add)
        xb.append(t)

    # ---------------- Q^T, K^T, V ----------------
    qkt = [persist.tile([128, N], f32, name=f"qkt{mb}") for mb in range(4)]
    for mb in range(4):
        for qc in range(2):
            ps = ps_s.tile([128, 512], f32, name="qk_ps", tag="ps_s")
            for c in range(2):
                nc.tensor.matmul(
                    ps,
                    lhsT=wt[c][:, mb * 128:(mb + 1) * 128].bitcast(MM_DT),
                    rhs=xt[c][:, qc * 512:(qc + 1) * 512].bitcast(MM_DT),
                    start=(c == 0), stop=(c == 1))
            nc.scalar.activation(qkt[mb][:, qc * 512:(qc + 1) * 512].bitcast(f32r),
                                 ps, Act.Identity, bias=bqk[mb])

    # V padded per head, fp16: [128, H, 65]; col 64 of each head slot is the
    # ones column that produces the softmax denominator in the AV matmul.
    vpad = [persist.tile([128, H, HD + 1], f16, name=f"vpad{kb}") for kb in range(KB)]
    ones4 = consts.tile([128, H], f16, name="ones4")
    nc.vector.memset(ones4, 1.0)
    for kb in range(KB):
        nc.vector.tensor_copy(
            vpad[kb][:, :, HD:HD + 1],
            ones4.rearrange("p (h o) -> p h o", o=1))
        ps = ps_o.tile([128, D], f32, name="v_ps", tag="ps_o")
        for c in range(2):
            nc.tensor.matmul(
                ps,
                lhsT=xt[c][:, kb * 128:(kb + 1) * 128].bitcast(MM_DT),
                rhs=wt[c][:, 2 * D:3 * D].bitcast(MM_DT),
                start=(c == 0), stop=(c == 1))
        nc.vector.tensor_tensor(
            vpad[kb][:, :, 0:HD],
            ps.rearrange("p (h e) -> p h e", h=H),
            bv_b.rearrange("p (h e) -> p h e", h=H),
            Alu.add)

    # ---------------- selection + bias build ----------------
    # biasf[q, k] = 0.02 * nd - 44 * [k not in top-32(q)]   (fp16, natural)
    # bias_t[kk, kb, q] = biasf[q, kb*128 + kk]             (DMA-transposed)
    bias_t = persist.tile([128, KB, N], f16, name="bias_t")

    for i in range(NT):
        drow = dstage.tile([128, N], f32, name="drow", tag="drow")
        dma_eng = nc.sync if i % 2 == 0 else nc.scalar
        dma_eng.dma_start(out=drow, in_=d_d[i * 128:(i + 1) * 128, :])
        nd = selp.tile([128, N], f32, name="nd", tag="nd")
        nc.scalar.activation(nd, drow, Act.Copy, scale=-1.0)  # nd = -d

        m32 = selp.tile([128, 32], f32, name="m32", tag="m32")
        sc = selp.tile([128, N], f32, name="selsc", tag="selsc")
        nc.vector.max(m32[:, 0:8], nd)
        nc.vector.match_replace(sc, m32[:, 0:8], nd, NEG_BIG)
        nc.vector.max(m32[:, 8:16], sc)
        nc.vector.match_replace(sc, m32[:, 8:16], sc, NEG_BIG)
        nc.vector.max(m32[:, 16:24], sc)
        nc.vector.match_replace(sc, m32[:, 16:24], sc, NEG_BIG)
        nc.vector.max(m32[:, 24:32], sc)
        nc.vector.match_replace(sc, m32[:, 24:32], sc, NEG_BIG)
        # sc == NEG_BIG exactly marks the reference top-32 multiset.

        # m40 = -44 where NOT selected, 0 where selected  (Pool)
        m40 = selp.tile([128, N], f32, name="m40", tag="m40")
        nc.gpsimd.tensor_scalar(m40, sc, 0.5 * NEG_BIG, MASK_M,
                                Alu.is_gt, Alu.mult)
        # biasf = 0.02*nd + m40  (fp16 out)
        biasf = selp.tile([128, N], f16, name="biasf", tag="biasf")
        nc.vector.scalar_tensor_tensor(
            out=biasf, in0=nd, scalar=1.0 / D_REF, in1=m40,
            op0=Alu.mult, op1=Alu.add)
        # transpose into bias_t columns i*128..(i+1)*128 (runs on DMA xbar)
        nc.sync.dma_start_transpose(
            out=bias_t[:, :, i * 128:(i + 1) * 128], in_=biasf)

    # ---------------- attention (transposed layout) ----------------
    # head h: Q^T/K^T rows live in qkt[mb], partitions (h%2)*64 .. +64
    attnt = [persist.tile([128, N], f16, name=f"attnt{c}") for c in range(2)]
    den4 = persist.tile([4, N], f32, name="den4")
    rd2 = [persist.tile([2, N], f32, name=f"rd2{c}") for c in range(2)]

    # chunk-outer: chunk c needs only bias tiles 2c, 2c+1, so attention starts
    # after two selection tiles and only the last 256-query chunk remains
    # un-overlapped after selection ends.
    CHUNKS = [(0, 512), (512, 256), (768, 256)]
    for q0, QW in CHUNKS:
        qs = slice(q0, q0 + QW)
        for h in range(H):
            qmb, kmb = h // 2, 2 + h // 2
            p0 = (h % 2) * HD
            pt_tiles = []
            for kb in range(KB):
                ptile = ptp.tile([128, QW], f16, name="pt", tag="pt")
                pt_tiles.append(ptile)
                ps = ps_s.tile([128, QW], f32, name="s_ps", tag="ps_s")
                nc.tensor.matmul(
                    ps,
                    lhsT=qkt[kmb][p0:p0 + HD, kb * 128:(kb + 1) * 128].bitcast(MM_DT),
                    rhs=qkt[qmb][p0:p0 + HD, qs].bitcast(MM_DT),
                    start=True, stop=False)
                nc.tensor.matmul(
                    ps, lhsT=identh, rhs=bias_t[:, kb, qs],
                    start=False, stop=True)
                nc.scalar.activation(ptile, ps, Act.Exp)
            av = ps_av.tile([HD + 1, QW], f32, name="av_ps", tag="ps_av")
            for kb in range(KB):
                nc.tensor.matmul(
                    av,
                    lhsT=vpad[kb][:, h, :],
                    rhs=pt_tiles[kb],
                    start=(kb == 0), stop=(kb == KB - 1))
            nc.scalar.activation(
                attnt[h // 2][(h % 2) * HD:(h % 2) * HD + HD, qs],
                av[0:HD, :], Act.Copy)
            nc.gpsimd.tensor_copy(den4[h:h + 1, qs], av[HD:HD + 1, :])

        # ---- normalize + epilogue for this query chunk
        for c in range(2):
            nc.vector.reciprocal(rd2[c][:, qs], den4[2 * c:2 * c + 2, qs])
            rbp = ps_rb.tile([128, QW], f32, name="rb_ps", tag="ps_rb")
            nc.tensor.matmul(rbp, lhsT=sel2.bitcast(MM_DT),
                             rhs=rd2[c][:, qs].bitcast(MM_DT))
            rbh = epi.tile([128, QW], f16, name="rbh", tag="rbh")
            nc.scalar.activation(rbh, rbp, Act.Copy)
            nc.vector.tensor_tensor(attnt[c][:, qs], attnt[c][:, qs], rbh,
                                    Alu.mult)

        for tb in range(q0 // 128, q0 // 128 + QW // 128):
            po = ps_o.tile([128, D], f32, name="o_ps", tag="ps_o")
            for c in range(2):
                nc.tensor.matmul(
                    po,
                    lhsT=attnt[c][:, tb * 128:(tb + 1) * 128],
                    rhs=wot[c],
                    start=(c == 0), stop=(c == 1))
            x = epi.tile([128, D], f32, name="x_epi", tag="x_epi")
            nc.gpsimd.tensor_tensor(x, po, xb[tb], Alu.add)
            st = epi.tile([128, 6], f32, name="st", tag="st")
            nc.vector.bn_stats(st, x)
            mv = epi.tile([128, 2], f32, name="mv", tag="mv")
            nc.vector.bn_aggr(mv, st)
            sd = epi.tile([128, 1], f32, name="sd", tag="sd")
            nc.scalar.activation(sd, mv[:, 1:2], Act.Sqrt, bias=epsc)
            rstd = epi.tile([128, 1], f32, name="rstd", tag="rstd")
            nc.vector.reciprocal(rstd, sd)
            xc = epi.tile([128, D], f32, name="xc_epi", tag="xc_epi")
            nc.gpsimd.tensor_scalar(xc, x, mv[:, 0:1], None, Alu.subtract)
            y = epi.tile([128, D], f32, name="y_epi", tag="y_epi")
            nc.vector.scalar_tensor_tensor(
                out=y, in0=g_b, scalar=rstd, in1=xc, op0=Alu.mult, op1=Alu.mult)
            nc.gpsimd.tensor_tensor(y, y, be_b, Alu.add)
            (nc.scalar if tb % 2 else nc.sync).dma_start(
                out=out_d[tb * 128:(tb + 1) * 128, :], in_=y)

    ctx.close()


_NC_CACHE = {}


def _get_nc(K: int):
    if K not in _NC_CACHE:
        _NC_CACHE[K] = build_nc(K)
    return _NC_CACHE[K]


def kernel(**inputs) -> np.ndarray:
    from concourse.bass_utils import run_bass_kernel_spmd

    K = int(np.asarray(inputs["K"]))
    assert K == 32, f"kernel specialized for K=32, got {K}"
    B = inputs["repr1"].shape[0]
    nc = _get_nc(K)

    shared = {
        "in_proj_w": np.ascontiguousarray(inputs["in_proj_w"], np.float32),
        "in_proj_b": np.ascontiguousarray(inputs["in_proj_b"], np.float32),
        "out_proj_w": np.ascontiguousarray(inputs["out_proj_w"], np.float32),
        "out_proj_b": np.ascontiguousarray(inputs["out_proj_b"], np.float32),
        "ln_gamma": np.ascontiguousarray(inputs["ln_gamma"], np.float32),
        "ln_beta": np.ascontiguousarray(inputs["ln_beta"], np.float32),
    }
    in_maps = []
    for b in range(B):
        m = dict(shared)
        m["repr1"] = np.ascontiguousarray(inputs["repr1"][b], np.float32)
        m["distances"] = np.ascontiguousarray(inputs["distances"][b], np.float32)
        in_maps.append(m)

    res = run_bass_kernel_spmd(nc, in_maps, list(range(B)))
    out = np.stack([np.asarray(res.results[b]["out"]) for b in range(B)])
    return out.astype(np.float32)
